# revision 17
# baseline (speedup 1.0000x reference)
"""Trainium2 Bass kernel for nn_DiagonalBiLSTM (PixelRNN-style diagonal BiLSTM).

8 NeuronCores, data-parallel over batch (2 images/core). Device layout is fully
"transposed": activations live as X^T [128ch, 16256pix] in SBUF with pixel
column t*128 + 64*b + m (t = diagonal 0..126, b = batch-in-core, m = row).

Per layer: LayerNorm is computed as xnhat = (x-mu)*rstd with gamma/beta folded
into the gate weights/biases. The 127-step diagonal LSTM scan runs both
directions at once: PSUM gate banks pack [L-f-lanes | R-f-lanes] on partitions;
block-diagonal zero-padded weight configs turn each recurrent contribution
(hprev and h, both directions) into one full-array matmul; the causal row-shift
of hprev is a column-offset read of the gap-layout h buffer. The L + row-
shifted-R combine accumulates in PSUM via two offset write patterns, and the
residual xnhat*gamma add is fused into the PSUM->SBUF scalar_tensor_tensor op.
"""

import functools
import os
import sys

import numpy as np

for _p in ("/opt/trn_rl_repo", "/root/.axon_site/_ro/trn_rl_repo"):
    if _p not in sys.path:
        sys.path.insert(0, _p)

import ml_dtypes  # noqa: E402
import concourse.bass as bass  # noqa: E402
import concourse.bacc as bacc  # noqa: E402
import concourse.tile as tile  # noqa: E402
from concourse import mybir  # noqa: E402
from concourse.bass_utils import run_bass_kernel_spmd  # noqa: E402

F32 = mybir.dt.float32
F32R = mybir.dt.float32r
I32 = mybir.dt.int32
BF16 = mybir.dt.bfloat16
AF = mybir.ActivationFunctionType
ALU = mybir.AluOpType
NPBF16 = ml_dtypes.bfloat16

B, M, NN = 16, 64, 64
F, H2, L = 64, 128, 2
T = M + NN - 1          # 127 diagonals
BPC = 2                 # batches per core
NCORES = 8
RB = BPC * M            # 128 rows per diagonal step
NPIX = T * RB           # 16256 pixel columns per core
EPS = 1e-6

_mask = np.ones((7, 7), np.float32)
_mask[3:, 4:] = 0.0
_mask[4:, :] = 0.0
_mask[3, 3] = 0.0
TAPS = [(i, j) for i in range(7) for j in range(7) if _mask[i, j]]
NT = len(TAPS)          # 24


# ---------------------------------------------------------------- host prep
def _skew_pad(im):
    """im [b, 64, 64, 1] -> padded skewed [b, 70, 133] bf16."""
    b = im.shape[0]
    out = np.zeros((b, M + 6, T + 6), np.float32)
    for i in range(M):
        out[:, 3 + i, 3 + i:3 + i + NN] = im[:, i, :, 0]
    return out.astype(NPBF16)


def _prep_params(inp):
    p = {}
    k = np.asarray(inp["conv_in_k"], np.float32) * _mask[:, :, None, None]
    p["convk"] = np.ascontiguousarray(
        np.stack([k[i, j, 0, :] for (i, j) in TAPS], 0)).astype(NPBF16)
    p["convbT"] = np.ascontiguousarray(
        np.asarray(inp["conv_in_b"], np.float32)[:, None])

    wis = np.asarray(inp["w_is"], np.float32)
    wss = np.asarray(inp["w_ss"], np.float32)
    woc = np.asarray(inp["w_oc"], np.float32)
    bis = np.asarray(inp["b_is"], np.float32)
    bss = np.asarray(inp["b_ss"], np.float32)
    boc = np.asarray(inp["b_oc"], np.float32)
    lns = np.asarray(inp["ln_s"], np.float32)
    lnb = np.asarray(inp["ln_b"], np.float32)
    h0 = np.asarray(inp["h0"], np.float32)

    # All SBUF-destined arrays are packed partition-first.
    wis_cfg = np.zeros((H2, L, 4, 128), np.float32)
    rec_cfg = np.zeros((H2, L, 8, 128), np.float32)    # idx = s01*4 + q
    woc_cfg = np.zeros((H2, L, H2), np.float32)
    gbiasT = np.zeros((4, L, H2), np.float32)
    resv = np.zeros((H2, L, 4), np.float32)            # gamL, cL, gamR, cR
    hinit = np.zeros((H2, L, 131), np.float32)
    for l in range(L):
        for d in range(2):
            sl = slice(64 * d, 64 * d + 64)
            wp = lns[l, d][:, None] * wis[l, d]
            bb = bis[l, d] + bss[l, d] + lnb[l, d] @ wis[l, d]
            for pg, g in enumerate((0, 1, 2, 3)):   # fg, ig, og, gg
                wis_cfg[:, l, pg, sl] = wp[:, 64 * g:64 * g + 64]
                gbiasT[pg, l, sl] = bb[64 * g:64 * g + 64]
                for s01 in range(2):
                    rec_cfg[sl, l, s01 * 4 + pg, sl] = \
                        wss[l, d, s01][:, 64 * g:64 * g + 64]
            woc_cfg[sl, l, :] = woc[l, d]
            resv[:, l, 2 * d + 0] = lns[l, d]
            resv[:, l, 2 * d + 1] = boc[l, d] + lnb[l, d]
            for blk in range(2):
                hinit[sl, l, 65 * blk + 1:65 * blk + 65] = h0[l, d].T
    p["wis_cfg"] = wis_cfg.astype(NPBF16)
    p["rec_cfg"] = rec_cfg.astype(NPBF16)
    p["woc_cfg"] = woc_cfg.astype(NPBF16)
    p["gbiasT"] = gbiasT.astype(NPBF16)
    p["resv"] = resv
    gind = np.zeros((4, 512), np.float32)
    for g in range(4):
        gind[g, 128 * g:128 * g + 128] = 1.0
    p["gind"] = gind.astype(NPBF16)
    p["hinit"] = hinit.astype(NPBF16)

    p["w1"] = np.asarray(inp["w_out1"], np.float32).astype(NPBF16)
    p["w2"] = np.asarray(inp["w_out2"], np.float32).astype(NPBF16)
    p["w3"] = np.asarray(inp["w_head"], np.float32).astype(NPBF16)
    hb = np.zeros((H2, 4), np.float32)
    hb[0:32, 0] = np.asarray(inp["b_out1"], np.float32)
    hb[0:32, 1] = np.asarray(inp["b_out2"], np.float32)
    hb[:, 2] = np.asarray(inp["b_head"], np.float32)[0:128]
    hb[:, 3] = np.asarray(inp["b_head"], np.float32)[128:256]
    p["headb"] = hb
    return p


def _dram_ap(handle, offset, dims):
    return bass.AP(tensor=handle, offset=offset, ap=[list(d) for d in dims])


def _blkview(t2d, off):
    """[128, W] -> [128, 2, 64] view of cols off + 65*k + j  (k<2, j<64)."""
    return t2d[:, off:off + 130].rearrange(
        "p (b c) -> p b c", b=2, c=65)[:, :, 0:64]


def _rview(t2d, off):
    """[128, W] -> [128, 2, 64] view of cols off + 64*k + j."""
    return t2d[:, off:off + 128].rearrange("p (b c) -> p b c", b=2, c=64)


DBG = False
TRACE = os.environ.get("BASS_KTRACE", "") == "1"
LAST_EXEC_NS = None
LAST_TRACE = None  # (insts, trace_path) when TRACE
PHASE_LIMIT = None  # None=all, else one of "conv","ln0","scan0","ln1","scan1"


@functools.cache
def _build(dbg=False, phase_limit=None, repeat=1):
    nc = bacc.Bacc("TRN2", target_bir_lowering=False, debug=False,
                   num_devices=NCORES)
    din = {}
    shapes = {
        "impad": ([BPC, M + 6, T + 6], BF16),
        "impadT": ([BPC, T + 6, M + 6], BF16),
        "convk": ([NT, H2], BF16), "convbT": ([H2, 1], F32),
        "wis_cfg": ([H2, L, 4, 128], BF16),
        "rec_cfg": ([H2, L, 8, 128], BF16),
        "woc_cfg": ([H2, L, H2], BF16),
        "gbiasT": ([4, L, H2], BF16),
        "gind": ([4, 512], BF16),
        "resv": ([H2, L, 4], F32),
        "hinit": ([H2, L, 131], BF16),
        "w1": ([H2, 32], BF16), "w2": ([32, 32], BF16), "w3": ([32, 256], BF16),
        "headb": ([H2, 4], F32),
    }
    for nm, (shp, dt) in shapes.items():
        din[nm] = nc.declare_dram_parameter(nm, shp, dt, isOutput=False)
    preds = nc.declare_dram_parameter("preds", [256, NPIX], F32, isOutput=True)
    dbg_t = {}
    if dbg:
        for nm in ("x0", "xnh0", "x1", "xnh1", "x2"):
            dbg_t[nm] = nc.declare_dram_parameter(nm, [H2, NPIX], F32,
                                                  isOutput=True)
    ab_bounce = nc.dram_tensor("ab_bounce", [2, NPIX], BF16)
    st_bounce = nc.dram_tensor("st_bounce", [2, NPIX], F32)

    from contextlib import ExitStack
    with tile.TileContext(nc) as tc, ExitStack() as ctx:
        big = ctx.enter_context(tc.tile_pool(name="big", bufs=1))
        XA = big.tile([H2, NPIX], BF16)
        XB = big.tile([H2, NPIX], BF16)
        XNH = big.tile([H2, NPIX], BF16)

        sg = ctx.enter_context(tc.tile_pool(name="singles", bufs=1))
        wis_sb = sg.tile([H2, L, 4, 128], BF16)
        rec_sb = sg.tile([H2, L, 8, 128], BF16)
        woc_sb = sg.tile([H2, L, H2], BF16)
        gbiasT_sb = sg.tile([4, L, H2], BF16)
        gind_sb = sg.tile([4, 512], BF16)
        resv_sb = sg.tile([H2, L, 4], F32)
        hinit_sb = sg.tile([H2, L, 131], BF16)
        w1_sb = sg.tile([H2, 32], BF16)
        w2_sb = sg.tile([32, 32], BF16)
        w3_sb = sg.tile([32, 256], BF16)
        headb_sb = sg.tile([H2, 4], F32)
        convk_sb = sg.tile([NT, H2], BF16)
        convbT_sb = sg.tile([H2, 1], F32)
        ones_f = sg.tile([1, 512], F32)
        ones_b = sg.tile([1, 512], BF16)
        ones_col = sg.tile([H2, 1], BF16)

        dma = nc.sync
        for nm, t in [("wis_cfg", wis_sb), ("rec_cfg", rec_sb),
                      ("woc_cfg", woc_sb), ("gbiasT", gbiasT_sb),
                      ("gind", gind_sb), ("resv", resv_sb),
                      ("hinit", hinit_sb), ("w1", w1_sb), ("w2", w2_sb),
                      ("w3", w3_sb), ("headb", headb_sb), ("convk", convk_sb),
                      ("convbT", convbT_sb)]:
            dma.dma_start(out=t[:], in_=din[nm].ap())
        nc.vector.memset(ones_f[:], 1.0)
        nc.vector.memset(ones_b[:], 1.0)
        nc.vector.memset(ones_col[:], 1.0)

        # ---------------------------------------------- conv -> XA (=X0)
        for _rep in range(repeat):
            _one_pass(nc, tc, ctx, dbg, dbg_t, din, preds, ab_bounce,
                      st_bounce, phase_limit,
                      XA, XB, XNH, wis_sb, rec_sb, woc_sb, gbiasT_sb,
                      gind_sb, resv_sb, hinit_sb,
                      w1_sb, w2_sb, w3_sb, headb_sb, convk_sb, convbT_sb,
                      ones_f, ones_b, ones_col)
    nc.compile()
    return nc


def _one_pass(nc, tc, ctx, dbg, dbg_t, din, preds, ab_bounce, st_bounce,
              phase_limit, XA, XB, XNH, wis_sb, rec_sb, woc_sb, gbiasT_sb,
              gind_sb, resv_sb, hinit_sb,
              w1_sb, w2_sb, w3_sb, headb_sb, convk_sb, convbT_sb,
              ones_f, ones_b, ones_col):
    if True:
        dma = nc.sync
        # ------------------------------------------ conv -> XA (=X0)
        with tc.tile_pool(name="patches", bufs=1) as ppool, \
             tc.tile_pool(name="convpsum", bufs=2, space="PSUM") as cpsum:
            pat = ppool.tile([NT, NPIX], BF16)
            for ti, (di, dj) in enumerate(TAPS):
                for bb in range(BPC):
                    (nc.gpsimd if (ti + bb) % 2 else nc.sync).dma_start(
                        out=pat[ti:ti + 1, :].rearrange(
                            "p (t m) -> p t m",
                            t=T, m=128)[:, :, 64 * bb:64 * bb + 64],
                        in_=_dram_ap(
                            din["impadT"],
                            bb * (M + 6) * (T + 6) + dj * (M + 6) + di,
                            [[0, 1], [M + 6, T], [1, M]]))
            for ci in range((NPIX + 511) // 512):
                s, n = ci * 512, min(512, NPIX - ci * 512)
                ps = cpsum.tile([H2, 512], F32, tag="cps")
                nc.tensor.matmul(ps[:, :n], convk_sb[:], pat[:, s:s + n],
                                 start=True, stop=True)
                if ci % 2 == 0:
                    nc.scalar.activation(XA[:, s:s + n], ps[:, :n],
                                         AF.Identity, bias=convbT_sb[:, 0:1])
                else:
                    nc.vector.tensor_scalar(XA[:, s:s + n], ps[:, :n],
                                            convbT_sb[:, 0:1], None, ALU.add)
        if dbg:
            _dump(nc, tc, XA, dbg_t["x0"])

        # ---------------------------------------------- layers
        stat = ctx.enter_context(tc.tile_pool(name="stats", bufs=1))
        stop_after = {"conv": 0, "ln0": 1, "scan0": 2, "ln1": 3,
                      "scan1": 4}.get(phase_limit, 99)
        for l in range(L):
            if stop_after <= 2 * l:
                break
            xin, xout = (XA, XB) if l == 0 else (XB, XA)
            _ln_phase(nc, tc, l, xin, XNH, xout, ones_col, stat, ab_bounce,
                      resv_sb)
            if dbg:
                _dump(nc, tc, XNH, dbg_t["xnh0" if l == 0 else "xnh1"])
            if stop_after <= 2 * l + 1:
                break
            _scan_phase(nc, tc, l, XNH, xout, wis_sb, rec_sb, woc_sb,
                        gbiasT_sb, gind_sb, hinit_sb)
            if dbg:
                _dump(nc, tc, xout, dbg_t["x1" if l == 0 else "x2"])
        if stop_after < 99:
            return

        # ---------------------------------------------- head
        with tc.tile_pool(name="hsb", bufs=3) as hsb, \
             tc.tile_pool(name="hpsum", bufs=2, space="PSUM") as hps:
            x2 = XA
            for ci in range((NPIX + 511) // 512):
                s, n = ci * 512, min(512, NPIX - ci * 512)
                p1 = hps.tile([32, 512], F32, tag="p1")
                nc.tensor.matmul(p1[:, :n], w1_sb[:], x2[:, s:s + n],
                                 start=True, stop=True)
                t1 = hsb.tile([32, 512], BF16, tag="t1")
                nc.scalar.activation(t1[:, :n], p1[:, :n], AF.Identity,
                                     bias=headb_sb[0:32, 0:1])
                p2 = hps.tile([32, 512], F32, tag="p2")
                nc.tensor.matmul(p2[:, :n], w2_sb[:], t1[:, :n],
                                 start=True, stop=True)
                t2 = hsb.tile([32, 512], BF16, tag="t2")
                nc.vector.tensor_scalar(t2[:, :n], p2[:, :n],
                                        headb_sb[0:32, 1:2], None, ALU.add)
                for hh in range(2):
                    p3 = hps.tile([128, 512], F32, tag=f"p3{hh}")
                    nc.tensor.matmul(p3[:, :n],
                                     w3_sb[:, 128 * hh:128 * hh + 128],
                                     t2[:, :n], start=True, stop=True)
                    s3 = hsb.tile([128, 512], F32, tag=f"s3{hh}")
                    if hh == 0:
                        nc.scalar.activation(s3[:, :n], p3[:, :n], AF.Identity,
                                             bias=headb_sb[:, 2 + hh:3 + hh])
                    else:
                        nc.vector.tensor_scalar(s3[:, :n], p3[:, :n],
                                                headb_sb[:, 2 + hh:3 + hh],
                                                None, ALU.add)
                    dma.dma_start(
                        out=preds.ap()[128 * hh:128 * hh + 128, s:s + n],
                        in_=s3[:, :n])


def _dump(nc, tc, src, ddst):
    with tc.tile_pool(name="dmp", bufs=2) as dp:
        for ci in range((NPIX + 511) // 512):
            s, n = ci * 512, min(512, NPIX - ci * 512)
            t = dp.tile([H2, 512], F32, tag="d")
            nc.vector.tensor_copy(t[:, :n], src[:, s:s + n])
            nc.sync.dma_start(out=ddst.ap()[:, s:s + n], in_=t[:, :n])


# ---------------------------------------------------------------- LN phase
def _ln_phase(nc, tc, l, X, XNH, XOUT, ones_col, stat, ab_bounce, resv_sb):
    """Per-pixel LayerNorm stats over ch (partitions) via ones-matmuls.

    Stats chunks are 508 px (= 4*127) so each chunk's [1, 508] sums DMA
    straight into 4 partition rows of the compact [128, 127] stat tiles."""
    CH = 508
    dma = nc.gpsimd
    with tc.tile_pool(name="lnp", bufs=4, space="PSUM") as lnp, \
         tc.tile_pool(name="lnsq", bufs=4) as lnsq, \
         tc.tile_pool(name="lnbc", bufs=2) as lnbc:
        mu = stat.tile([H2, T], F32, tag="mu")
        msq = stat.tile([H2, T], F32, tag="msq")
        for ci in range(NPIX // CH):
            s = ci * CH
            sq = lnsq.tile([H2, CH], BF16, tag="sq")
            if ci % 2 == 0:
                nc.scalar.square(sq[:], X[:, s:s + CH])
            else:
                nc.vector.tensor_mul(sq[:], X[:, s:s + CH], X[:, s:s + CH])
            pp = lnp.tile([1, CH], F32, tag="pp")
            nc.tensor.matmul(pp[0:1, :], ones_col[:], X[:, s:s + CH],
                             start=True, stop=True)
            qq = lnp.tile([1, CH], F32, tag="qq")
            nc.tensor.matmul(qq[0:1, :], ones_col[:], sq[:],
                             start=True, stop=True)
            sr = lnsq.tile([1, CH], F32, tag="sr")
            qr = lnsq.tile([1, CH], F32, tag="qr")
            nc.vector.tensor_copy(sr[0:1, :], pp[0:1, :])
            nc.scalar.copy(qr[0:1, :], qq[0:1, :])
            dma.dma_start(out=mu[4 * ci:4 * ci + 4, :], in_=sr[0:1, :])
            dma.dma_start(out=msq[4 * ci:4 * ci + 4, :], in_=qr[0:1, :])
        nc.vector.tensor_scalar_mul(mu[:], mu[:], 1.0 / H2)
        var = stat.tile([H2, T], F32, tag="var")
        nc.vector.tensor_mul(var[:], mu[:], mu[:])
        nc.vector.scalar_tensor_tensor(var[:], msq[:], 1.0 / H2, var[:],
                                       ALU.mult, ALU.subtract)
        nc.vector.tensor_scalar_add(var[:], var[:], EPS)
        # rstd = rsqrt(var): magic initial guess + 3 Newton iterations (DVE)
        y = stat.tile([H2, T], F32, tag="y")
        nc.vector.tensor_scalar(y[:].bitcast(I32), var[:].bitcast(I32),
                                1, None, ALU.logical_shift_right)
        nc.vector.tensor_scalar(y[:].bitcast(I32), y[:].bitcast(I32),
                                0x5F3759DF, None, ALU.subtract)
        nc.vector.tensor_scalar_mul(y[:].bitcast(I32), y[:].bitcast(I32), -1)
        half = stat.tile([H2, T], F32, tag="half")
        nc.vector.tensor_scalar_mul(half[:], var[:], -0.5)
        t0 = stat.tile([H2, T], F32, tag="t0")
        for _ in range(3):
            nc.vector.tensor_mul(t0[:], y[:], y[:])
            nc.vector.tensor_mul(t0[:], t0[:], half[:])
            nc.vector.tensor_scalar_add(t0[:], t0[:], 1.5)
            nc.vector.tensor_mul(y[:], y[:], t0[:])
        arow = stat.tile([H2, T], BF16, tag="arow")
        brow = stat.tile([H2, T], BF16, tag="brow")
        nc.vector.tensor_copy(arow[:], y[:])
        nc.vector.scalar_tensor_tensor(brow[:], mu[:], -1.0, y[:],
                                       ALU.mult, ALU.mult)
        dma.dma_start(out=_dram_ap(ab_bounce, 0, [[127, 128], [1, 127]]),
                      in_=arow[:])
        dma.dma_start(out=_dram_ap(ab_bounce, NPIX, [[127, 128], [1, 127]]),
                      in_=brow[:])
        # xnhat = X*a + b with a/b broadcast across partitions via
        # stride-0-partition DMA reads from DRAM; then the scan-independent
        # residual XOUT = xnh*gamL + cL (+ rowshifted xnh*gamR + cR).
        BC = 2048
        for ci in range((NPIX + BC - 1) // BC):
            s = ci * BC
            n = min(BC, NPIX - s)
            nb = n // 128
            abc = lnbc.tile([H2, BC], BF16, tag="abc")
            bbc = lnbc.tile([H2, BC], BF16, tag="bbc")
            dma.dma_start(out=abc[:, :n],
                          in_=_dram_ap(ab_bounce, s, [[0, H2], [1, n]]))
            nc.sync.dma_start(out=bbc[:, :n],
                              in_=_dram_ap(ab_bounce, NPIX + s, [[0, H2], [1, n]]))
            tmp = lnbc.tile([H2, BC], BF16, tag="tmp")
            nc.vector.tensor_mul(tmp[:, :n], X[:, s:s + n], abc[:, :n])
            nc.vector.tensor_add(XNH[:, s:s + n], tmp[:, :n], bbc[:, :n])
            nc.vector.tensor_scalar(XOUT[:, s:s + n], XNH[:, s:s + n],
                                    resv_sb[:, l, 0:1], resv_sb[:, l, 1:2],
                                    ALU.mult, ALU.add)
            tb = lnbc.tile([H2, BC], BF16, tag="tb")
            nc.gpsimd.tensor_scalar(tb[:, :n], XNH[:, s:s + n],
                                    resv_sb[:, l, 2:3], resv_sb[:, l, 3:4],
                                    ALU.mult, ALU.add)
            xv = XOUT[:, s:s + n].rearrange("p (t c) -> p t c", t=2 * nb,
                                            c=64)
            bv = tb[:, :n].rearrange("p (t c) -> p t c", t=2 * nb, c=64)
            nc.vector.tensor_add(xv[:, :, 1:64], xv[:, :, 1:64],
                                 bv[:, :, 0:63])


# ---------------------------------------------------------------- scan
def _scan_phase(nc, tc, l, XNH, XOUT, wis_sb, rec_sb, woc_sb,
                gbiasT_sb, gind_sb, hinit_sb):
    with tc.tile_pool(name=f"sc{l}", bufs=1) as sp, \
         tc.tile_pool(name=f"zp{l}", bufs=3, space="PSUM") as zp, \
         tc.tile_pool(name=f"np{l}", bufs=2, space="PSUM") as nxp, \
         tc.tile_pool(name=f"nr{l}", bufs=2, space="PSUM") as nxr, \
         tc.tile_pool(name=f"gt{l}", bufs=3) as gt:
        H = sp.tile([H2, 131], BF16)
        C = sp.tile([H2, 128], F32)
        nc.sync.dma_start(out=H[:], in_=hinit_sb[:, l, :])
        nc.vector.memset(C[:], 0.0)
        hprev_v = _blkview(H, 0)
        hdata_v = _blkview(H, 1)

        # Z layout per step: [fg | ig | og | gg] 128-col blocks
        def prestage(Zt, t):
            tl, tr = t, T - 1 - t
            nc.tensor.matmul(Zt[:], gbiasT_sb[:, l, :], gind_sb[:],
                             start=True, stop=False)
            for g in range(4):
                zg = Zt[:, 128 * g:128 * g + 128]
                for d, tc_ in ((0, tl), (1, tr)):
                    nc.tensor.matmul(
                        zg[64 * d:64 * d + 64, :],
                        wis_sb[:, l, g, 64 * d:64 * d + 64],
                        XNH[:, 128 * tc_:128 * tc_ + 128],
                        start=False, stop=False)

        def pn_finish(t):
            # woc matmuls consume h(t) which is current H contents
            PNL = nxp.tile([H2, 128], F32, tag="PNL")
            nc.tensor.matmul(PNL[:], woc_sb[0:64, l, :], hdata_v[0:64],
                             start=True, stop=True)
            PNR = nxr.tile([H2, 132], F32, tag="PNR")
            for blk in range(2):
                o1, o2 = 65 * blk + 1, 65 * blk + 2
                nc.tensor.matmul(PNR[:, o2:o2 + 64], woc_sb[64:128, l, :],
                                 H[64:128, o1:o1 + 64], start=True,
                                 stop=True)
            return PNL, PNR

        def x_write(PNL, PNR, t):
            tl, tr = t, T - 1 - t
            xl = XOUT[:, 128 * tl:128 * tl + 128]
            nc.vector.tensor_add(xl, xl, PNL[:])
            xr = _rview(XOUT[:, 128 * tr:128 * tr + 128], 0)
            pr = _blkview(PNR, 1)
            nc.vector.tensor_add(xr[:, :, 1:64], xr[:, :, 1:64],
                                 pr[:, :, 1:64])

        Zcur = zp.tile([H2, 512], F32, tag="Z")
        prestage(Zcur, 0)
        pend = None          # (PNL, PNR, t) awaiting X write
        for t in range(T):
            Z = Zcur
            # ---- h(t-1)-dependent matmuls FIRST in the PE queue, so the
            # recurrence starts the moment h(t-1) lands; gg first so
            # tanh(gg) overlaps the remaining gate matmuls
            for g in (3, 0, 1, 2):
                zg = Z[:, 128 * g:128 * g + 128]
                nc.tensor.matmul(zg, rec_sb[:, l, 0 + g, :], hprev_v,
                                 start=False, stop=False)
                nc.tensor.matmul(zg, rec_sb[:, l, 4 + g, :], hdata_v,
                                 start=False, stop=(g == 2))
            if pend is not None:
                x_write(*pn_finish(pend), pend)
            # next step's x-path prestage fills the PE while the gate
            # chain runs on ACT/DVE
            if t + 1 < T:
                Zcur = zp.tile([H2, 512], F32, tag="Z")
                prestage(Zcur, t + 1)
            # ---- gates: sigmoid(fg|ig|og) contiguous, tanh(gg)
            tg = gt.tile([H2, 128], BF16, tag="tg")
            nc.scalar.activation(tg[:], Z[:, 384:512], AF.Tanh)
            sfio = gt.tile([H2, 384], BF16, tag="sfio")
            nc.scalar.activation(sfio[:], Z[:, 0:384], AF.Sigmoid)
            t2 = gt.tile([H2, 128], F32, tag="t2")
            nc.vector.tensor_mul(t2[:], sfio[:, 0:128], C[:])
            t1 = gt.tile([H2, 128], F32, tag="t1")
            nc.vector.tensor_mul(t1[:], sfio[:, 128:256], tg[:])
            nc.vector.tensor_add(C[:], t1[:], t2[:])
            tcn = gt.tile([H2, 128], BF16, tag="tcn")
            nc.scalar.activation(tcn[:], C[:], AF.Tanh)
            nc.vector.tensor_mul(
                hdata_v, _rview(sfio[:, 256:384], 0), _rview(tcn, 0))
            pend = t
        x_write(*pn_finish(pend), pend)


# ---------------------------------------------------------------- runner
def kernel(**inputs):
    p = _prep_params(inputs)
    im = np.asarray(inputs["im"], np.float32)
    nc = _build(DBG)
    in_maps = []
    for c in range(NCORES):
        m = dict(p)
        m["impad"] = _skew_pad(im[BPC * c:BPC * c + BPC])
        m["impadT"] = np.ascontiguousarray(m["impad"].transpose(0, 2, 1))
        in_maps.append(m)
    kw = {}
    if TRACE:
        import tempfile
        kw = dict(trace=True, tmpdir=tempfile.mkdtemp(prefix="ktrace_"))
    res = run_bass_kernel_spmd(nc, in_maps, core_ids=list(range(NCORES)), **kw)
    global LAST_EXEC_NS, LAST_TRACE
    if res.exec_time_ns is not None:
        LAST_EXEC_NS = res.exec_time_ns
        LAST_TRACE = res.instructions_and_trace
    out = np.zeros((B, M, NN, 256), np.float32)
    mi, ni = np.meshgrid(np.arange(M), np.arange(NN), indexing="ij")
    cols = (mi + ni) * RB + mi
    for c in range(NCORES):
        pr = res.results[c]["preds"]
        for bb in range(BPC):
            out[BPC * c + bb] = pr[:, cols + 64 * bb].transpose(1, 2, 0)
    return out



# revision 18
# speedup vs baseline: 1.0292x; 1.0292x over previous
"""Trainium2 Bass kernel for nn_DiagonalBiLSTM (PixelRNN-style diagonal BiLSTM).

8 NeuronCores, data-parallel over batch (2 images/core). Device layout is fully
"transposed": activations live as X^T [128ch, 16256pix] in SBUF with pixel
column t*128 + 64*b + m (t = diagonal 0..126, b = batch-in-core, m = row).

Per layer: LayerNorm is computed as xnhat = (x-mu)*rstd with gamma/beta folded
into the gate weights/biases. The 127-step diagonal LSTM scan runs both
directions at once: PSUM gate banks pack [L-f-lanes | R-f-lanes] on partitions;
block-diagonal zero-padded weight configs turn each recurrent contribution
(hprev and h, both directions) into one full-array matmul; the causal row-shift
of hprev is a column-offset read of the gap-layout h buffer. The L + row-
shifted-R combine accumulates in PSUM via two offset write patterns, and the
residual xnhat*gamma add is fused into the PSUM->SBUF scalar_tensor_tensor op.
"""

import functools
import os
import sys

import numpy as np

for _p in ("/opt/trn_rl_repo", "/root/.axon_site/_ro/trn_rl_repo"):
    if _p not in sys.path:
        sys.path.insert(0, _p)

import ml_dtypes  # noqa: E402
import concourse.bass as bass  # noqa: E402
import concourse.bacc as bacc  # noqa: E402
import concourse.tile as tile  # noqa: E402
from concourse import mybir  # noqa: E402
from concourse.bass_utils import run_bass_kernel_spmd  # noqa: E402

F32 = mybir.dt.float32
F32R = mybir.dt.float32r
I32 = mybir.dt.int32
BF16 = mybir.dt.bfloat16
AF = mybir.ActivationFunctionType
ALU = mybir.AluOpType
NPBF16 = ml_dtypes.bfloat16

B, M, NN = 16, 64, 64
F, H2, L = 64, 128, 2
T = M + NN - 1          # 127 diagonals
BPC = 2                 # batches per core
NCORES = 8
RB = BPC * M            # 128 rows per diagonal step
NPIX = T * RB           # 16256 pixel columns per core
EPS = 1e-6

_mask = np.ones((7, 7), np.float32)
_mask[3:, 4:] = 0.0
_mask[4:, :] = 0.0
_mask[3, 3] = 0.0
TAPS = [(i, j) for i in range(7) for j in range(7) if _mask[i, j]]
NT = len(TAPS)          # 24


# ---------------------------------------------------------------- host prep
def _skew_pad(im):
    """im [b, 64, 64, 1] -> padded skewed [b, 70, 133] bf16."""
    b = im.shape[0]
    out = np.zeros((b, M + 6, T + 6), np.float32)
    for i in range(M):
        out[:, 3 + i, 3 + i:3 + i + NN] = im[:, i, :, 0]
    return out.astype(NPBF16)


def _prep_params(inp):
    p = {}
    k = np.asarray(inp["conv_in_k"], np.float32) * _mask[:, :, None, None]
    p["convk"] = np.ascontiguousarray(
        np.stack([k[i, j, 0, :] for (i, j) in TAPS], 0)).astype(NPBF16)
    p["convbT"] = np.ascontiguousarray(
        np.asarray(inp["conv_in_b"], np.float32)[:, None])

    wis = np.asarray(inp["w_is"], np.float32)
    wss = np.asarray(inp["w_ss"], np.float32)
    woc = np.asarray(inp["w_oc"], np.float32)
    bis = np.asarray(inp["b_is"], np.float32)
    bss = np.asarray(inp["b_ss"], np.float32)
    boc = np.asarray(inp["b_oc"], np.float32)
    lns = np.asarray(inp["ln_s"], np.float32)
    lnb = np.asarray(inp["ln_b"], np.float32)
    h0 = np.asarray(inp["h0"], np.float32)

    # All SBUF-destined arrays are packed partition-first.
    wis_cfg = np.zeros((H2, L, 4, 128), np.float32)
    rec_cfg = np.zeros((H2, L, 8, 128), np.float32)    # idx = s01*4 + q
    woc_cfg = np.zeros((H2, L, H2), np.float32)
    gbiasT = np.zeros((4, L, H2), np.float32)
    resv = np.zeros((H2, L, 4), np.float32)            # gamL, cL, gamR, cR
    hinit = np.zeros((H2, L, 131), np.float32)
    for l in range(L):
        for d in range(2):
            sl = slice(64 * d, 64 * d + 64)
            wp = lns[l, d][:, None] * wis[l, d]
            bb = bis[l, d] + bss[l, d] + lnb[l, d] @ wis[l, d]
            for pg, g in enumerate((0, 1, 2, 3)):   # fg, ig, og, gg
                wis_cfg[:, l, pg, sl] = wp[:, 64 * g:64 * g + 64]
                gbiasT[pg, l, sl] = bb[64 * g:64 * g + 64]
                for s01 in range(2):
                    rec_cfg[sl, l, s01 * 4 + pg, sl] = \
                        wss[l, d, s01][:, 64 * g:64 * g + 64]
            woc_cfg[sl, l, :] = woc[l, d]
            resv[:, l, 2 * d + 0] = lns[l, d]
            resv[:, l, 2 * d + 1] = boc[l, d] + lnb[l, d]
            for blk in range(2):
                hinit[sl, l, 65 * blk + 1:65 * blk + 65] = h0[l, d].T
    p["wis_cfg"] = wis_cfg.astype(NPBF16)
    p["rec_cfg"] = rec_cfg.astype(NPBF16)
    p["woc_cfg"] = woc_cfg.astype(NPBF16)
    p["gbiasT"] = gbiasT.astype(NPBF16)
    p["resv"] = resv
    gind = np.zeros((4, 512), np.float32)
    for g in range(4):
        gind[g, 128 * g:128 * g + 128] = 1.0
    p["gind"] = gind.astype(NPBF16)
    p["hinit"] = hinit.astype(NPBF16)

    p["w1"] = np.asarray(inp["w_out1"], np.float32).astype(NPBF16)
    p["w2"] = np.asarray(inp["w_out2"], np.float32).astype(NPBF16)
    p["w3"] = np.asarray(inp["w_head"], np.float32).astype(NPBF16)
    hb = np.zeros((H2, 4), np.float32)
    hb[0:32, 0] = np.asarray(inp["b_out1"], np.float32)
    hb[0:32, 1] = np.asarray(inp["b_out2"], np.float32)
    hb[:, 2] = np.asarray(inp["b_head"], np.float32)[0:128]
    hb[:, 3] = np.asarray(inp["b_head"], np.float32)[128:256]
    p["headb"] = hb
    return p


def _dram_ap(handle, offset, dims):
    return bass.AP(tensor=handle, offset=offset, ap=[list(d) for d in dims])


def _blkview(t2d, off):
    """[128, W] -> [128, 2, 64] view of cols off + 65*k + j  (k<2, j<64)."""
    return t2d[:, off:off + 130].rearrange(
        "p (b c) -> p b c", b=2, c=65)[:, :, 0:64]


def _rview(t2d, off):
    """[128, W] -> [128, 2, 64] view of cols off + 64*k + j."""
    return t2d[:, off:off + 128].rearrange("p (b c) -> p b c", b=2, c=64)


DBG = False
TRACE = os.environ.get("BASS_KTRACE", "") == "1"
LAST_EXEC_NS = None
LAST_TRACE = None  # (insts, trace_path) when TRACE
PHASE_LIMIT = None  # None=all, else one of "conv","ln0","scan0","ln1","scan1"


@functools.cache
def _build(dbg=False, phase_limit=None, repeat=1):
    nc = bacc.Bacc("TRN2", target_bir_lowering=False, debug=False,
                   num_devices=NCORES)
    din = {}
    shapes = {
        "impad": ([BPC, M + 6, T + 6], BF16),
        "impadT": ([BPC, T + 6, M + 6], BF16),
        "convk": ([NT, H2], BF16), "convbT": ([H2, 1], F32),
        "wis_cfg": ([H2, L, 4, 128], BF16),
        "rec_cfg": ([H2, L, 8, 128], BF16),
        "woc_cfg": ([H2, L, H2], BF16),
        "gbiasT": ([4, L, H2], BF16),
        "gind": ([4, 512], BF16),
        "resv": ([H2, L, 4], F32),
        "hinit": ([H2, L, 131], BF16),
        "w1": ([H2, 32], BF16), "w2": ([32, 32], BF16), "w3": ([32, 256], BF16),
        "headb": ([H2, 4], F32),
    }
    for nm, (shp, dt) in shapes.items():
        din[nm] = nc.declare_dram_parameter(nm, shp, dt, isOutput=False)
    preds = nc.declare_dram_parameter("preds", [256, NPIX], F32, isOutput=True)
    dbg_t = {}
    if dbg:
        for nm in ("x0", "xnh0", "x1", "xnh1", "x2"):
            dbg_t[nm] = nc.declare_dram_parameter(nm, [H2, NPIX], F32,
                                                  isOutput=True)
    ab_bounce = nc.dram_tensor("ab_bounce", [2, NPIX], BF16)
    st_bounce = nc.dram_tensor("st_bounce", [2, NPIX], F32)

    from contextlib import ExitStack
    with tile.TileContext(nc) as tc, ExitStack() as ctx:
        big = ctx.enter_context(tc.tile_pool(name="big", bufs=1))
        XA = big.tile([H2, NPIX], BF16)
        XB = big.tile([H2, NPIX], BF16)
        XNH = big.tile([H2, NPIX], BF16)

        sg = ctx.enter_context(tc.tile_pool(name="singles", bufs=1))
        wis_sb = sg.tile([H2, L, 4, 128], BF16)
        rec_sb = sg.tile([H2, L, 8, 128], BF16)
        woc_sb = sg.tile([H2, L, H2], BF16)
        gbiasT_sb = sg.tile([4, L, H2], BF16)
        gind_sb = sg.tile([4, 512], BF16)
        resv_sb = sg.tile([H2, L, 4], F32)
        hinit_sb = sg.tile([H2, L, 131], BF16)
        w1_sb = sg.tile([H2, 32], BF16)
        w2_sb = sg.tile([32, 32], BF16)
        w3_sb = sg.tile([32, 256], BF16)
        headb_sb = sg.tile([H2, 4], F32)
        convk_sb = sg.tile([NT, H2], BF16)
        convbT_sb = sg.tile([H2, 1], F32)
        ones_f = sg.tile([1, 512], F32)
        ones_b = sg.tile([1, 512], BF16)
        ones_col = sg.tile([H2, 1], BF16)

        dma = nc.sync
        for nm, t in [("wis_cfg", wis_sb), ("rec_cfg", rec_sb),
                      ("woc_cfg", woc_sb), ("gbiasT", gbiasT_sb),
                      ("gind", gind_sb), ("resv", resv_sb),
                      ("hinit", hinit_sb), ("w1", w1_sb), ("w2", w2_sb),
                      ("w3", w3_sb), ("headb", headb_sb), ("convk", convk_sb),
                      ("convbT", convbT_sb)]:
            dma.dma_start(out=t[:], in_=din[nm].ap())
        nc.vector.memset(ones_f[:], 1.0)
        nc.vector.memset(ones_b[:], 1.0)
        nc.vector.memset(ones_col[:], 1.0)

        # ---------------------------------------------- conv -> XA (=X0)
        for _rep in range(repeat):
            _one_pass(nc, tc, ctx, dbg, dbg_t, din, preds, ab_bounce,
                      st_bounce, phase_limit,
                      XA, XB, XNH, wis_sb, rec_sb, woc_sb, gbiasT_sb,
                      gind_sb, resv_sb, hinit_sb,
                      w1_sb, w2_sb, w3_sb, headb_sb, convk_sb, convbT_sb,
                      ones_f, ones_b, ones_col)
    nc.compile()
    return nc


def _one_pass(nc, tc, ctx, dbg, dbg_t, din, preds, ab_bounce, st_bounce,
              phase_limit, XA, XB, XNH, wis_sb, rec_sb, woc_sb, gbiasT_sb,
              gind_sb, resv_sb, hinit_sb,
              w1_sb, w2_sb, w3_sb, headb_sb, convk_sb, convbT_sb,
              ones_f, ones_b, ones_col):
    if True:
        dma = nc.sync
        # ------------------------------------------ conv -> XA (=X0)
        with tc.tile_pool(name="patches", bufs=1) as ppool, \
             tc.tile_pool(name="convpsum", bufs=2, space="PSUM") as cpsum:
            pat = ppool.tile([NT, NPIX], BF16)
            for ti, (di, dj) in enumerate(TAPS):
                for bb in range(BPC):
                    (nc.gpsimd if (ti + bb) % 2 else nc.sync).dma_start(
                        out=pat[ti:ti + 1, :].rearrange(
                            "p (t m) -> p t m",
                            t=T, m=128)[:, :, 64 * bb:64 * bb + 64],
                        in_=_dram_ap(
                            din["impadT"],
                            bb * (M + 6) * (T + 6) + dj * (M + 6) + di,
                            [[0, 1], [M + 6, T], [1, M]]))
            for ci in range((NPIX + 511) // 512):
                s, n = ci * 512, min(512, NPIX - ci * 512)
                ps = cpsum.tile([H2, 512], F32, tag="cps")
                nc.tensor.matmul(ps[:, :n], convk_sb[:], pat[:, s:s + n],
                                 start=True, stop=True)
                if ci % 2 == 0:
                    nc.scalar.activation(XA[:, s:s + n], ps[:, :n],
                                         AF.Identity, bias=convbT_sb[:, 0:1])
                else:
                    nc.vector.tensor_scalar(XA[:, s:s + n], ps[:, :n],
                                            convbT_sb[:, 0:1], None, ALU.add)
        if dbg:
            _dump(nc, tc, XA, dbg_t["x0"])

        # ---------------------------------------------- layers
        stat = ctx.enter_context(tc.tile_pool(name="stats", bufs=1))
        stop_after = {"conv": 0, "ln0": 1, "scan0": 2, "ln1": 3,
                      "scan1": 4}.get(phase_limit, 99)
        for l in range(L):
            if stop_after <= 2 * l:
                break
            xin, xout = (XA, XB) if l == 0 else (XB, XA)
            _ln_phase(nc, tc, l, xin, XNH, xout, ones_col, stat, ab_bounce,
                      resv_sb)
            if dbg:
                _dump(nc, tc, XNH, dbg_t["xnh0" if l == 0 else "xnh1"])
            if stop_after <= 2 * l + 1:
                break
            _scan_phase(nc, tc, l, XNH, xout, wis_sb, rec_sb, woc_sb,
                        gbiasT_sb, gind_sb, hinit_sb)
            if dbg:
                _dump(nc, tc, xout, dbg_t["x1" if l == 0 else "x2"])
        if stop_after < 99:
            return

        # ---------------------------------------------- head
        with tc.tile_pool(name="hsb", bufs=3) as hsb, \
             tc.tile_pool(name="hpsum", bufs=2, space="PSUM") as hps:
            x2 = XA
            for ci in range((NPIX + 511) // 512):
                s, n = ci * 512, min(512, NPIX - ci * 512)
                p1 = hps.tile([32, 512], F32, tag="p1")
                nc.tensor.matmul(p1[:, :n], w1_sb[:], x2[:, s:s + n],
                                 start=True, stop=True)
                t1 = hsb.tile([32, 512], BF16, tag="t1")
                nc.scalar.activation(t1[:, :n], p1[:, :n], AF.Identity,
                                     bias=headb_sb[0:32, 0:1])
                p2 = hps.tile([32, 512], F32, tag="p2")
                nc.tensor.matmul(p2[:, :n], w2_sb[:], t1[:, :n],
                                 start=True, stop=True)
                t2 = hsb.tile([32, 512], BF16, tag="t2")
                nc.vector.tensor_scalar(t2[:, :n], p2[:, :n],
                                        headb_sb[0:32, 1:2], None, ALU.add)
                for hh in range(2):
                    p3 = hps.tile([128, 512], F32, tag=f"p3{hh}")
                    nc.tensor.matmul(p3[:, :n],
                                     w3_sb[:, 128 * hh:128 * hh + 128],
                                     t2[:, :n], start=True, stop=True)
                    s3 = hsb.tile([128, 512], F32, tag=f"s3{hh}")
                    if hh == 0:
                        nc.scalar.activation(s3[:, :n], p3[:, :n], AF.Identity,
                                             bias=headb_sb[:, 2 + hh:3 + hh])
                    else:
                        nc.vector.tensor_scalar(s3[:, :n], p3[:, :n],
                                                headb_sb[:, 2 + hh:3 + hh],
                                                None, ALU.add)
                    dma.dma_start(
                        out=preds.ap()[128 * hh:128 * hh + 128, s:s + n],
                        in_=s3[:, :n])


def _dump(nc, tc, src, ddst):
    with tc.tile_pool(name="dmp", bufs=2) as dp:
        for ci in range((NPIX + 511) // 512):
            s, n = ci * 512, min(512, NPIX - ci * 512)
            t = dp.tile([H2, 512], F32, tag="d")
            nc.vector.tensor_copy(t[:, :n], src[:, s:s + n])
            nc.sync.dma_start(out=ddst.ap()[:, s:s + n], in_=t[:, :n])


# ---------------------------------------------------------------- LN phase
def _ln_phase(nc, tc, l, X, XNH, XOUT, ones_col, stat, ab_bounce, resv_sb):
    """Per-pixel LayerNorm stats over ch (partitions) via ones-matmuls.

    Stats chunks are 508 px (= 4*127) so each chunk's [1, 508] sums DMA
    straight into 4 partition rows of the compact [128, 127] stat tiles."""
    CH = 508
    dma = nc.gpsimd
    with tc.tile_pool(name="lnp", bufs=4, space="PSUM") as lnp, \
         tc.tile_pool(name="lnsq", bufs=4) as lnsq, \
         tc.tile_pool(name="lnbc", bufs=2) as lnbc:
        mu = stat.tile([H2, T], F32, tag="mu")
        msq = stat.tile([H2, T], F32, tag="msq")
        for ci in range(NPIX // CH):
            s = ci * CH
            sq = lnsq.tile([H2, CH], BF16, tag="sq")
            if ci % 2 == 0:
                nc.scalar.square(sq[:], X[:, s:s + CH])
            else:
                nc.vector.tensor_mul(sq[:], X[:, s:s + CH], X[:, s:s + CH])
            pp = lnp.tile([1, CH], F32, tag="pp")
            nc.tensor.matmul(pp[0:1, :], ones_col[:], X[:, s:s + CH],
                             start=True, stop=True)
            qq = lnp.tile([1, CH], F32, tag="qq")
            nc.tensor.matmul(qq[0:1, :], ones_col[:], sq[:],
                             start=True, stop=True)
            sr = lnsq.tile([1, CH], F32, tag="sr")
            qr = lnsq.tile([1, CH], F32, tag="qr")
            nc.vector.tensor_copy(sr[0:1, :], pp[0:1, :])
            nc.scalar.copy(qr[0:1, :], qq[0:1, :])
            dma.dma_start(out=mu[4 * ci:4 * ci + 4, :], in_=sr[0:1, :])
            dma.dma_start(out=msq[4 * ci:4 * ci + 4, :], in_=qr[0:1, :])
        nc.vector.tensor_scalar_mul(mu[:], mu[:], 1.0 / H2)
        var = stat.tile([H2, T], F32, tag="var")
        nc.vector.tensor_mul(var[:], mu[:], mu[:])
        nc.vector.scalar_tensor_tensor(var[:], msq[:], 1.0 / H2, var[:],
                                       ALU.mult, ALU.subtract)
        nc.vector.tensor_scalar_add(var[:], var[:], EPS)
        # rstd = rsqrt(var): magic initial guess + 3 Newton iterations (DVE)
        y = stat.tile([H2, T], F32, tag="y")
        nc.vector.tensor_scalar(y[:].bitcast(I32), var[:].bitcast(I32),
                                1, None, ALU.logical_shift_right)
        nc.vector.tensor_scalar(y[:].bitcast(I32), y[:].bitcast(I32),
                                0x5F3759DF, None, ALU.subtract)
        nc.vector.tensor_scalar_mul(y[:].bitcast(I32), y[:].bitcast(I32), -1)
        half = stat.tile([H2, T], F32, tag="half")
        nc.vector.tensor_scalar_mul(half[:], var[:], -0.5)
        t0 = stat.tile([H2, T], F32, tag="t0")
        for _ in range(3):
            nc.vector.tensor_mul(t0[:], y[:], y[:])
            nc.vector.tensor_mul(t0[:], t0[:], half[:])
            nc.vector.tensor_scalar_add(t0[:], t0[:], 1.5)
            nc.vector.tensor_mul(y[:], y[:], t0[:])
        arow = stat.tile([H2, T], BF16, tag="arow")
        brow = stat.tile([H2, T], BF16, tag="brow")
        nc.vector.tensor_copy(arow[:], y[:])
        nc.vector.scalar_tensor_tensor(brow[:], mu[:], -1.0, y[:],
                                       ALU.mult, ALU.mult)
        dma.dma_start(out=_dram_ap(ab_bounce, 0, [[127, 128], [1, 127]]),
                      in_=arow[:])
        dma.dma_start(out=_dram_ap(ab_bounce, NPIX, [[127, 128], [1, 127]]),
                      in_=brow[:])
        # xnhat = X*a + b with a/b broadcast across partitions via
        # stride-0-partition DMA reads from DRAM; then the scan-independent
        # residual XOUT = xnh*gamL + cL (+ rowshifted xnh*gamR + cR).
        BC = 2048
        for ci in range((NPIX + BC - 1) // BC):
            s = ci * BC
            n = min(BC, NPIX - s)
            nb = n // 128
            abc = lnbc.tile([H2, BC], BF16, tag="abc")
            bbc = lnbc.tile([H2, BC], BF16, tag="bbc")
            dma.dma_start(out=abc[:, :n],
                          in_=_dram_ap(ab_bounce, s, [[0, H2], [1, n]]))
            nc.sync.dma_start(out=bbc[:, :n],
                              in_=_dram_ap(ab_bounce, NPIX + s, [[0, H2], [1, n]]))
            tmp = lnbc.tile([H2, BC], BF16, tag="tmp")
            nc.vector.tensor_mul(tmp[:, :n], X[:, s:s + n], abc[:, :n])
            nc.vector.tensor_add(XNH[:, s:s + n], tmp[:, :n], bbc[:, :n])
            nc.vector.tensor_scalar(XOUT[:, s:s + n], XNH[:, s:s + n],
                                    resv_sb[:, l, 0:1], resv_sb[:, l, 1:2],
                                    ALU.mult, ALU.add)
            tb = lnbc.tile([H2, BC], BF16, tag="tb")
            nc.gpsimd.tensor_scalar(tb[:, :n], XNH[:, s:s + n],
                                    resv_sb[:, l, 2:3], resv_sb[:, l, 3:4],
                                    ALU.mult, ALU.add)
            xv = XOUT[:, s:s + n].rearrange("p (t c) -> p t c", t=2 * nb,
                                            c=64)
            bv = tb[:, :n].rearrange("p (t c) -> p t c", t=2 * nb, c=64)
            nc.vector.tensor_add(xv[:, :, 1:64], xv[:, :, 1:64],
                                 bv[:, :, 0:63])


# ---------------------------------------------------------------- scan
def _scan_phase(nc, tc, l, XNH, XOUT, wis_sb, rec_sb, woc_sb,
                gbiasT_sb, gind_sb, hinit_sb):
    with tc.tile_pool(name=f"sc{l}", bufs=1) as sp, \
         tc.tile_pool(name=f"zp{l}", bufs=3, space="PSUM") as zp, \
         tc.tile_pool(name=f"np{l}", bufs=2, space="PSUM") as nxp, \
         tc.tile_pool(name=f"nr{l}", bufs=2, space="PSUM") as nxr, \
         tc.tile_pool(name=f"gt{l}", bufs=3) as gt:
        H = sp.tile([H2, 131], BF16)
        C = sp.tile([H2, 128], F32)
        nc.sync.dma_start(out=H[:], in_=hinit_sb[:, l, :])
        nc.vector.memset(C[:], 0.0)
        hprev_v = _blkview(H, 0)
        hdata_v = _blkview(H, 1)

        # Z layout per step: [fg | ig | og | gg] 128-col blocks
        def prestage(Zt, t):
            tl, tr = t, T - 1 - t
            nc.tensor.matmul(Zt[:], gbiasT_sb[:, l, :], gind_sb[:],
                             start=True, stop=False)
            for g in range(4):
                zg = Zt[:, 128 * g:128 * g + 128]
                for d, tc_ in ((0, tl), (1, tr)):
                    nc.tensor.matmul(
                        zg[64 * d:64 * d + 64, :],
                        wis_sb[:, l, g, 64 * d:64 * d + 64],
                        XNH[:, 128 * tc_:128 * tc_ + 128],
                        start=False, stop=False)

        def pn_finish(t):
            # woc matmuls consume h(t) which is current H contents
            PNL = nxp.tile([H2, 128], F32, tag="PNL")
            nc.tensor.matmul(PNL[:], woc_sb[0:64, l, :], hdata_v[0:64],
                             start=True, stop=True)
            PNR = nxr.tile([H2, 132], F32, tag="PNR")
            for blk in range(2):
                o1, o2 = 65 * blk + 1, 65 * blk + 2
                nc.tensor.matmul(PNR[:, o2:o2 + 64], woc_sb[64:128, l, :],
                                 H[64:128, o1:o1 + 64], start=True,
                                 stop=True)
            return PNL, PNR

        def x_write(PNL, PNR, t):
            tl, tr = t, T - 1 - t
            xl = XOUT[:, 128 * tl:128 * tl + 128]
            nc.vector.tensor_add(xl, xl, PNL[:])
            xr = _rview(XOUT[:, 128 * tr:128 * tr + 128], 0)
            pr = _blkview(PNR, 1)
            nc.vector.tensor_add(xr[:, :, 1:64], xr[:, :, 1:64],
                                 pr[:, :, 1:64])

        Zcur = zp.tile([H2, 512], F32, tag="Z")
        prestage(Zcur, 0)
        pend = None          # (PNL, PNR, t) awaiting X write
        for t in range(T):
            Z = Zcur
            # ---- h(t-1)-dependent matmuls FIRST in the PE queue, so the
            # recurrence starts the moment h(t-1) lands; fg/ig first so
            # sigmoid(fg|ig) can begin before the gg/og matmuls finish
            for g in (0, 1, 3, 2):
                zg = Z[:, 128 * g:128 * g + 128]
                nc.tensor.matmul(zg, rec_sb[:, l, 0 + g, :], hprev_v,
                                 start=False, stop=False)
                nc.tensor.matmul(zg, rec_sb[:, l, 4 + g, :], hdata_v,
                                 start=False, stop=(g == 2))
            # woc consumes h(t-1) and must precede this step's h update;
            # the DVE x-writes are deferred past the gate chain
            if pend is not None:
                pnl_p, pnr_p = pn_finish(pend)
            # next step's x-path prestage fills the PE while the gate
            # chain runs on ACT/DVE
            if t + 1 < T:
                Zcur = zp.tile([H2, 512], F32, tag="Z")
                prestage(Zcur, t + 1)
            # ---- gates: sigmoid(fg|ig) on the chain; og off-chain
            sfi = gt.tile([H2, 256], BF16, tag="sfi")
            nc.scalar.activation(sfi[:], Z[:, 0:256], AF.Sigmoid)
            tg = gt.tile([H2, 128], BF16, tag="tg")
            nc.scalar.activation(tg[:], Z[:, 384:512], AF.Tanh)
            so = gt.tile([H2, 128], BF16, tag="so")
            nc.scalar.activation(so[:], Z[:, 256:384], AF.Sigmoid)
            t2 = gt.tile([H2, 128], F32, tag="t2")
            nc.vector.tensor_mul(t2[:], sfi[:, 0:128], C[:])
            t1 = gt.tile([H2, 128], F32, tag="t1")
            nc.vector.tensor_mul(t1[:], sfi[:, 128:256], tg[:])
            nc.vector.tensor_add(C[:], t1[:], t2[:])
            tcn = gt.tile([H2, 128], BF16, tag="tcn")
            nc.scalar.activation(tcn[:], C[:], AF.Tanh)
            nc.vector.tensor_mul(
                hdata_v, _rview(so, 0), _rview(tcn, 0))
            if pend is not None:
                x_write(pnl_p, pnr_p, pend)
            pend = t
        x_write(*pn_finish(pend), pend)


# ---------------------------------------------------------------- runner
def kernel(**inputs):
    p = _prep_params(inputs)
    im = np.asarray(inputs["im"], np.float32)
    nc = _build(DBG)
    in_maps = []
    for c in range(NCORES):
        m = dict(p)
        m["impad"] = _skew_pad(im[BPC * c:BPC * c + BPC])
        m["impadT"] = np.ascontiguousarray(m["impad"].transpose(0, 2, 1))
        in_maps.append(m)
    kw = {}
    if TRACE:
        import tempfile
        kw = dict(trace=True, tmpdir=tempfile.mkdtemp(prefix="ktrace_"))
    res = run_bass_kernel_spmd(nc, in_maps, core_ids=list(range(NCORES)), **kw)
    global LAST_EXEC_NS, LAST_TRACE
    if res.exec_time_ns is not None:
        LAST_EXEC_NS = res.exec_time_ns
        LAST_TRACE = res.instructions_and_trace
    out = np.zeros((B, M, NN, 256), np.float32)
    mi, ni = np.meshgrid(np.arange(M), np.arange(NN), indexing="ij")
    cols = (mi + ni) * RB + mi
    for c in range(NCORES):
        pr = res.results[c]["preds"]
        for bb in range(BPC):
            out[BPC * c + bb] = pr[:, cols + 64 * bb].transpose(1, 2, 0)
    return out

